# revision 4
# baseline (speedup 1.0000x reference)
"""CrossAttention kernel for 8 TRN2 NeuronCores.

Sharding: core c handles batch b = c//2 and head-group g = c%2 (4 of 8 heads,
a 256-wide slice of the inner dim). Each core computes a partial output
projection; the host sums the two head-group partials per batch (bo is fed
only to g==0 cores so it is added exactly once).

Per-core dataflow (all matmuls in float32r — full PE rate, ~1.6e-4 rel err;
f32r operands must be DVE/ACT-produced and live at partition base 0):
  x (4096,320)   --PE transpose, 512-col blocks-->  xT block (320,512)
  qT_h (64,4096) = Wq_h^T x^T       (per-head tiles, partition base 0)
  ctx (1024,768) --PE transpose-->  ctxT blocks ->  kT_h (64,1024), v
  v3[m] (128, 4head x 128) = [v_h | ones]  (ones-columns give softmax sums)
  per (head, 1024-wide n-block):
    scT (128m,1024n) = kT_h^T qT_h   (k=64)       -> PSUM
    expT = exp(SCALE * scT)                        -> SBUF (ACT, f32r)
    ov (128,1024) += v3[m]^T expT   over 8 m-chunks; rows 64:128 = sums
    outT_h = ov[0:64] * recip(ov[64:128])          -> SBUF (DVE)
  out (4096,320) = outT^T Wo + bo   (DVE bias add)
"""

import numpy as np
import concourse.bass as bass
import concourse.tile as tile
from concourse import bacc, masks, mybir
from concourse.bass_utils import run_bass_kernel_spmd

F32 = mybir.dt.float32
F32R = mybir.dt.float32r

N = 4096  # query length
M = 1024  # context length
QD = 320  # query feature dim
CD = 768  # context feature dim
HD = 256  # per-core inner dim (4 heads x 64)
DH = 64  # head dim
NHL = 4  # heads per core
E = 320  # output dim
SCALE = DH**-0.5

NB = 1024  # attention n-block
N_NB = N // NB
MCH = M // 128  # 8 m-chunks

_NC_CACHE = {}


def build_nc():
    nc = bacc.Bacc("TRN2", target_bir_lowering=False, debug=False)
    x_d = nc.dram_tensor("x", [N, QD], F32, kind="ExternalInput").ap()
    ctx_d = nc.dram_tensor("ctx", [M, CD], F32, kind="ExternalInput").ap()
    wq_d = nc.dram_tensor("wq", [QD, HD], F32, kind="ExternalInput").ap()
    wk_d = nc.dram_tensor("wk", [CD, HD], F32, kind="ExternalInput").ap()
    wv_d = nc.dram_tensor("wv", [CD, HD], F32, kind="ExternalInput").ap()
    wo_d = nc.dram_tensor("wo", [HD, E], F32, kind="ExternalInput").ap()
    bo_d = nc.dram_tensor("bo", [E], F32, kind="ExternalInput").ap()
    out_d = nc.dram_tensor("out", [N, E], F32, kind="ExternalOutput").ap()

    # c-chunk layout of the contraction dims
    QCH = [(0, 128), (128, 128), (256, 64)]  # QD = 320
    CCH = [(128 * i, 128) for i in range(6)]  # CD = 768

    with tile.TileContext(nc) as tc:
        with tc.tile_pool(name="persist", bufs=1) as pp:
            ident = pp.tile([128, 128], F32, tag="ident")
            masks.make_identity(nc, ident[:])

            bo_b = pp.tile([128, E], F32, tag="bo_b")
            nc.gpsimd.dma_start(
                out=bo_b[:],
                in_=bass.AP(tensor=bo_d.tensor, offset=0, ap=[[0, 128], [1, E]]),
            )

            # ---- weights: DMA fp32 staging -> DVE round to f32r
            wq_t, wk_t, wv_t, wo_t = [], [], [], []
            with tc.tile_pool(name="wstage", bufs=3) as ws:
                for ci, (c0, cw) in enumerate(QCH):
                    st = ws.tile([128, HD], F32, tag="wst")
                    nc.gpsimd.dma_start(out=st[:cw, :], in_=wq_d[c0 : c0 + cw, :])
                    t = pp.tile([cw, HD], F32R, tag=f"wq{ci}", name=f"wq{ci}")
                    nc.vector.tensor_copy(t[:], st[:cw, :])
                    wq_t.append(t)
                for ci, (c0, cw) in enumerate(CCH):
                    st = ws.tile([128, HD], F32, tag="wst")
                    nc.gpsimd.dma_start(out=st[:], in_=wk_d[c0 : c0 + cw, :])
                    t = pp.tile([128, HD], F32R, tag=f"wk{ci}", name=f"wk{ci}")
                    nc.vector.tensor_copy(t[:], st[:])
                    wk_t.append(t)
                for ci, (c0, cw) in enumerate(CCH):
                    st = ws.tile([128, HD], F32, tag="wst")
                    nc.gpsimd.dma_start(out=st[:], in_=wv_d[c0 : c0 + cw, :])
                    t = pp.tile([128, HD], F32R, tag=f"wv{ci}", name=f"wv{ci}")
                    nc.vector.tensor_copy(t[:], st[:])
                    wv_t.append(t)
                for j in range(2):
                    st = ws.tile([128, E], F32, tag="wost")
                    nc.gpsimd.dma_start(out=st[:], in_=wo_d[128 * j : 128 * (j + 1), :])
                    t = pp.tile([128, E], F32R, tag=f"wo{j}", name=f"wo{j}")
                    nc.vector.tensor_copy(t[:], st[:])
                    wo_t.append(t)

            ones_f = pp.tile([128, NHL, DH], F32, tag="ones_f")
            nc.vector.memset(ones_f[:], 1.0)
            vones = pp.tile([128, NHL, DH], F32R, tag="vones")
            nc.vector.tensor_copy(vones[:], ones_f[:])

            qT = [pp.tile([DH, N], F32R, tag=f"qT{h}", name=f"qT{h}") for h in range(NHL)]
            kT = [pp.tile([DH, M], F32R, tag=f"kT{h}", name=f"kT{h}") for h in range(NHL)]
            v3 = [
                pp.tile([128, NHL, 128], F32R, tag=f"v3_{m}", name=f"v3_{m}")
                for m in range(MCH)
            ]

            # ---- phase A: blockwise transpose + q/k/v projections
            with (
                tc.tile_pool(name="stage", bufs=3) as stg,
                tc.tile_pool(name="blk", bufs=2) as blk,
                tc.tile_pool(name="tpp", bufs=3, space="PSUM") as tpp,
                tc.tile_pool(name="qkp", bufs=2, space="PSUM") as qkp,
                tc.tile_pool(name="vvp", bufs=2, space="PSUM") as vvp,
            ):
                # x -> xT blocks -> qT
                for nb in range(N // 512):
                    xtb = [
                        blk.tile([cw, 512], F32R, tag=f"xtb{ci}", name=f"xtb{ci}")
                        for ci, (c0, cw) in enumerate(QCH)
                    ]
                    for i in range(4):
                        st = stg.tile([128, QD], F32, tag="xst")
                        r0 = 512 * nb + 128 * i
                        nc.gpsimd.dma_start(out=st[:], in_=x_d[r0 : r0 + 128, :])
                        for ci, (c0, cw) in enumerate(QCH):
                            pt = tpp.tile([128, 128], F32, tag="tp")
                            nc.tensor.transpose(pt[:cw, :], st[:, c0 : c0 + cw], ident[:])
                            nc.vector.tensor_copy(
                                xtb[ci][:, 128 * i : 128 * (i + 1)], pt[:cw, :]
                            )
                    for j in range(2):
                        ps = qkp.tile([128, 512], F32, tag="qps")
                        for ci, (c0, cw) in enumerate(QCH):
                            nc.tensor.matmul(
                                ps[:],
                                wq_t[ci][:, 128 * j : 128 * (j + 1)],
                                xtb[ci][:cw, :],
                                start=(ci == 0),
                                stop=(ci == len(QCH) - 1),
                            )
                        nc.vector.tensor_copy(
                            qT[2 * j][:, 512 * nb : 512 * (nb + 1)], ps[0:64, :]
                        )
                        nc.vector.tensor_copy(
                            qT[2 * j + 1][:, 512 * nb : 512 * (nb + 1)], ps[64:128, :]
                        )

                # ctx -> ctxT blocks -> kT, v3
                for mb in range(M // 512):
                    ctb = [
                        blk.tile([128, 512], F32R, tag=f"ctb{ci}", name=f"ctb{ci}")
                        for ci in range(6)
                    ]
                    for i in range(4):
                        st = stg.tile([128, CD], F32, tag="cst")
                        r0 = 512 * mb + 128 * i
                        nc.gpsimd.dma_start(out=st[:], in_=ctx_d[r0 : r0 + 128, :])
                        for ci, (c0, cw) in enumerate(CCH):
                            pt = tpp.tile([128, 128], F32, tag="tp")
                            nc.tensor.transpose(pt[:, :], st[:, c0 : c0 + cw], ident[:])
                            nc.vector.tensor_copy(
                                ctb[ci][:, 128 * i : 128 * (i + 1)], pt[:, :]
                            )
                    for j in range(2):
                        ps = qkp.tile([128, 512], F32, tag="qps")
                        for ci in range(6):
                            nc.tensor.matmul(
                                ps[:],
                                wk_t[ci][:, 128 * j : 128 * (j + 1)],
                                ctb[ci][:, :],
                                start=(ci == 0),
                                stop=(ci == 5),
                            )
                        nc.vector.tensor_copy(
                            kT[2 * j][:, 512 * mb : 512 * (mb + 1)], ps[0:64, :]
                        )
                        nc.vector.tensor_copy(
                            kT[2 * j + 1][:, 512 * mb : 512 * (mb + 1)], ps[64:128, :]
                        )
                    for mc in range(4):
                        m = 4 * mb + mc
                        ps = vvp.tile([128, HD], F32, tag="vps")
                        for ci in range(6):
                            nc.tensor.matmul(
                                ps[:],
                                ctb[ci][:, 128 * mc : 128 * (mc + 1)],
                                wv_t[ci][:],
                                start=(ci == 0),
                                stop=(ci == 5),
                            )
                        nc.vector.tensor_copy(
                            v3[m][:, :, 0:DH],
                            ps[:].rearrange("p (h d) -> p h d", h=NHL),
                        )
                        nc.vector.tensor_copy(v3[m][:, :, DH:128], vones[:])

            # ---- attention
            outT = [
                pp.tile([128, N], F32R, tag=f"outT{j}", name=f"outT{j}") for j in range(2)
            ]
            with (
                tc.tile_pool(name="scp", bufs=2, space="PSUM") as scp,
                tc.tile_pool(name="ovp", bufs=2, space="PSUM") as ovp,
                tc.tile_pool(name="expp", bufs=1) as expp,
                tc.tile_pool(name="recp", bufs=2) as recp,
            ):
                for h in range(NHL):
                    for nb in range(N_NB):
                        n0 = NB * nb
                        expT = [
                            expp.tile([128, NB], F32R, tag=f"exp{m}", name=f"exp{m}")
                            for m in range(MCH)
                        ]
                        for m in range(MCH):
                            sc = scp.tile([128, NB], F32, tag="sc")
                            for j in range(NB // 512):
                                nc.tensor.matmul(
                                    sc[:, 512 * j : 512 * (j + 1)],
                                    kT[h][:, 128 * m : 128 * (m + 1)],
                                    qT[h][:, n0 + 512 * j : n0 + 512 * (j + 1)],
                                    start=True,
                                    stop=True,
                                )
                            nc.scalar.activation(
                                expT[m][:],
                                sc[:],
                                mybir.ActivationFunctionType.Exp,
                                scale=SCALE,
                            )
                        ov = ovp.tile([128, NB], F32, tag="ov")
                        for m in range(MCH):
                            for j in range(NB // 512):
                                nc.tensor.matmul(
                                    ov[:, 512 * j : 512 * (j + 1)],
                                    v3[m][:, h, :],
                                    expT[m][:, 512 * j : 512 * (j + 1)],
                                    start=(m == 0),
                                    stop=(m == MCH - 1),
                                )
                        rc = recp.tile([64, NB], F32, tag="rc")
                        nc.vector.reciprocal(rc[:], ov[64:128, :])
                        nc.vector.tensor_mul(
                            outT[h // 2][64 * (h % 2) : 64 * (h % 2) + 64, n0 : n0 + NB],
                            ov[0:64, :],
                            rc[:],
                        )

            # ---- output projection + bias
            with (
                tc.tile_pool(name="fop", bufs=4, space="PSUM") as fop,
                tc.tile_pool(name="fot", bufs=4) as fotp,
            ):
                for i in range(N // 128):
                    fo = fop.tile([128, E], F32, tag="fo")
                    for j in range(2):
                        nc.tensor.matmul(
                            fo[:],
                            outT[j][:, 128 * i : 128 * (i + 1)],
                            wo_t[j][:],
                            start=(j == 0),
                            stop=(j == 1),
                        )
                    ft = fotp.tile([128, E], F32, tag="ft")
                    nc.vector.tensor_add(ft[:], fo[:], bo_b[:])
                    nc.sync.dma_start(out=out_d[128 * i : 128 * (i + 1), :], in_=ft[:])

    nc.compile()
    return nc


def _get_nc():
    if "nc" not in _NC_CACHE:
        _NC_CACHE["nc"] = build_nc()
    return _NC_CACHE["nc"]


def _in_maps(x, context, Wq, Wk, Wv, Wo, bo):
    x = np.asarray(x, dtype=np.float32)
    context = np.asarray(context, dtype=np.float32)
    Wq = np.asarray(Wq, dtype=np.float32)
    Wk = np.asarray(Wk, dtype=np.float32)
    Wv = np.asarray(Wv, dtype=np.float32)
    Wo = np.asarray(Wo, dtype=np.float32)
    bo = np.asarray(bo, dtype=np.float32)
    zeros_bo = np.zeros_like(bo)
    maps = []
    for c in range(8):
        b, g = c // 2, c % 2
        sl = slice(HD * g, HD * (g + 1))
        maps.append(
            {
                "x": np.ascontiguousarray(x[b]),
                "ctx": np.ascontiguousarray(context[b]),
                "wq": np.ascontiguousarray(Wq[:, sl]),
                "wk": np.ascontiguousarray(Wk[:, sl]),
                "wv": np.ascontiguousarray(Wv[:, sl]),
                "wo": np.ascontiguousarray(Wo[sl, :]),
                "bo": bo if g == 0 else zeros_bo,
            }
        )
    return maps


def run_spmd(inputs, trace=False):
    nc = _get_nc()
    maps = _in_maps(**inputs)
    return run_bass_kernel_spmd(nc, maps, core_ids=list(range(8)), trace=trace)


def kernel(x, context, Wq, Wk, Wv, Wo, bo):
    res = run_spmd(dict(x=x, context=context, Wq=Wq, Wk=Wk, Wv=Wv, Wo=Wo, bo=bo))
    B = np.asarray(x).shape[0]
    out = np.empty((B, N, E), dtype=np.float32)
    for b in range(B):
        out[b] = res.results[2 * b]["out"] + res.results[2 * b + 1]["out"]
    return out


# revision 5
# speedup vs baseline: 156.2370x; 156.2370x over previous
"""CrossAttention kernel for 8 TRN2 NeuronCores.

Sharding: core c handles batch b = c//2 and head-group g = c%2 (4 of 8 heads,
a 256-wide slice of the inner dim). Each core computes a partial output
projection; the host sums the two head-group partials per batch (bo is fed
only to g==0 cores so it is added exactly once).

Per-core dataflow (all matmuls in float32r — full PE rate, ~1.6e-4 rel err;
f32r operands must be DVE/ACT-produced and live at partition base 0):
  x (4096,320)   --PE transpose, 512-col blocks-->  xT block (320,512)
  qT_h (64,4096) = Wq_h^T x^T       (per-head tiles, partition base 0)
  ctx (1024,768) --PE transpose-->  ctxT blocks ->  kT_h (64,1024), v
  v3[m] (128, 4head x 128) = [v_h | ones]  (ones-columns give softmax sums)
  per (head, 1024-wide n-block):
    scT (128m,1024n) = kT_h^T qT_h   (k=64)       -> PSUM
    expT = exp(SCALE * scT)                        -> SBUF (ACT, f32r)
    ov (128,1024) += v3[m]^T expT   over 8 m-chunks; rows 64:128 = sums
    outT_h = ov[0:64] * recip(ov[64:128])          -> SBUF (DVE)
  out (4096,320) = outT^T Wo + bo   (DVE bias add)
"""

import numpy as np
import concourse.bass as bass
import concourse.tile as tile
from concourse import bacc, masks, mybir
from concourse.bass_utils import run_bass_kernel_spmd

F32 = mybir.dt.float32
F32R = mybir.dt.float32r

N = 4096  # query length
M = 1024  # context length
QD = 320  # query feature dim
CD = 768  # context feature dim
HD = 256  # per-core inner dim (4 heads x 64)
DH = 64  # head dim
NHL = 4  # heads per core
E = 320  # output dim
SCALE = DH**-0.5

NB = 1024  # attention n-block
N_NB = N // NB
MCH = M // 128  # 8 m-chunks

_NC_CACHE = {}


def build_nc(reps=1):
    nc = bacc.Bacc("TRN2", target_bir_lowering=False, debug=False)
    x_d = nc.dram_tensor("x", [N, QD], F32, kind="ExternalInput").ap()
    ctx_d = nc.dram_tensor("ctx", [M, CD], F32, kind="ExternalInput").ap()
    wq_d = nc.dram_tensor("wq", [QD, HD], F32, kind="ExternalInput").ap()
    wk_d = nc.dram_tensor("wk", [CD, HD], F32, kind="ExternalInput").ap()
    wv_d = nc.dram_tensor("wv", [CD, HD], F32, kind="ExternalInput").ap()
    wo_d = nc.dram_tensor("wo", [HD, E], F32, kind="ExternalInput").ap()
    bo_d = nc.dram_tensor("bo", [E], F32, kind="ExternalInput").ap()
    out_d = nc.dram_tensor("out", [N, E], F32, kind="ExternalOutput").ap()

    # c-chunk layout of the contraction dims
    QCH = [(0, 128), (128, 128), (256, 64)]  # QD = 320
    CCH = [(128 * i, 128) for i in range(6)]  # CD = 768

    with tile.TileContext(nc) as tc:
      for _rep in range(reps):
        with tc.tile_pool(name="persist", bufs=1) as pp:
            ident = pp.tile([128, 128], F32, tag="ident")
            masks.make_identity(nc, ident[:])

            bo_b = pp.tile([128, E], F32, tag="bo_b")
            nc.gpsimd.dma_start(
                out=bo_b[:],
                in_=bass.AP(tensor=bo_d.tensor, offset=0, ap=[[0, 128], [1, E]]),
            )

            # ---- weights: DMA fp32 staging -> DVE round to f32r
            wq_t, wk_t, wv_t, wo_t = [], [], [], []
            with tc.tile_pool(name="wstage", bufs=3) as ws:
                for ci, (c0, cw) in enumerate(QCH):
                    st = ws.tile([128, HD], F32, tag="wst")
                    nc.gpsimd.dma_start(out=st[:cw, :], in_=wq_d[c0 : c0 + cw, :])
                    t = pp.tile([cw, HD], F32R, tag=f"wq{ci}", name=f"wq{ci}")
                    nc.vector.tensor_copy(t[:], st[:cw, :])
                    wq_t.append(t)
                for ci, (c0, cw) in enumerate(CCH):
                    st = ws.tile([128, HD], F32, tag="wst")
                    nc.gpsimd.dma_start(out=st[:], in_=wk_d[c0 : c0 + cw, :])
                    t = pp.tile([128, HD], F32R, tag=f"wk{ci}", name=f"wk{ci}")
                    nc.vector.tensor_copy(t[:], st[:])
                    wk_t.append(t)
                for ci, (c0, cw) in enumerate(CCH):
                    st = ws.tile([128, HD], F32, tag="wst")
                    nc.gpsimd.dma_start(out=st[:], in_=wv_d[c0 : c0 + cw, :])
                    t = pp.tile([128, HD], F32R, tag=f"wv{ci}", name=f"wv{ci}")
                    nc.vector.tensor_copy(t[:], st[:])
                    wv_t.append(t)
                for j in range(2):
                    st = ws.tile([128, E], F32, tag="wost")
                    nc.gpsimd.dma_start(out=st[:], in_=wo_d[128 * j : 128 * (j + 1), :])
                    t = pp.tile([128, E], F32R, tag=f"wo{j}", name=f"wo{j}")
                    nc.vector.tensor_copy(t[:], st[:])
                    wo_t.append(t)

            ones_f = pp.tile([128, NHL, DH], F32, tag="ones_f")
            nc.vector.memset(ones_f[:], 1.0)
            vones = pp.tile([128, NHL, DH], F32R, tag="vones")
            nc.vector.tensor_copy(vones[:], ones_f[:])

            qT = [pp.tile([DH, N], F32R, tag=f"qT{h}", name=f"qT{h}") for h in range(NHL)]
            kT = [pp.tile([DH, M], F32R, tag=f"kT{h}", name=f"kT{h}") for h in range(NHL)]
            v3 = [
                pp.tile([128, NHL, 128], F32R, tag=f"v3_{m}", name=f"v3_{m}")
                for m in range(MCH)
            ]

            # ---- phase A: blockwise transpose + q/k/v projections
            with (
                tc.tile_pool(name="stage", bufs=3) as stg,
                tc.tile_pool(name="blk", bufs=2) as blk,
                tc.tile_pool(name="tpp", bufs=3, space="PSUM") as tpp,
                tc.tile_pool(name="qkp", bufs=2, space="PSUM") as qkp,
                tc.tile_pool(name="vvp", bufs=2, space="PSUM") as vvp,
            ):
                # x -> xT blocks -> qT
                for nb in range(N // 512):
                    xtb = [
                        blk.tile([cw, 512], F32R, tag=f"xtb{ci}", name=f"xtb{ci}")
                        for ci, (c0, cw) in enumerate(QCH)
                    ]
                    for i in range(4):
                        st = stg.tile([128, QD], F32, tag="xst")
                        r0 = 512 * nb + 128 * i
                        nc.gpsimd.dma_start(out=st[:], in_=x_d[r0 : r0 + 128, :])
                        for ci, (c0, cw) in enumerate(QCH):
                            pt = tpp.tile([128, 128], F32, tag="tp")
                            nc.tensor.transpose(pt[:cw, :], st[:, c0 : c0 + cw], ident[:])
                            nc.vector.tensor_copy(
                                xtb[ci][:, 128 * i : 128 * (i + 1)], pt[:cw, :]
                            )
                    for j in range(2):
                        ps = qkp.tile([128, 512], F32, tag="qps")
                        for ci, (c0, cw) in enumerate(QCH):
                            nc.tensor.matmul(
                                ps[:],
                                wq_t[ci][:, 128 * j : 128 * (j + 1)],
                                xtb[ci][:cw, :],
                                start=(ci == 0),
                                stop=(ci == len(QCH) - 1),
                            )
                        nc.vector.tensor_copy(
                            qT[2 * j][:, 512 * nb : 512 * (nb + 1)], ps[0:64, :]
                        )
                        nc.vector.tensor_copy(
                            qT[2 * j + 1][:, 512 * nb : 512 * (nb + 1)], ps[64:128, :]
                        )

                # ctx -> ctxT blocks -> kT, v3
                for mb in range(M // 512):
                    ctb = [
                        blk.tile([128, 512], F32R, tag=f"ctb{ci}", name=f"ctb{ci}")
                        for ci in range(6)
                    ]
                    for i in range(4):
                        st = stg.tile([128, CD], F32, tag="cst")
                        r0 = 512 * mb + 128 * i
                        nc.gpsimd.dma_start(out=st[:], in_=ctx_d[r0 : r0 + 128, :])
                        for ci, (c0, cw) in enumerate(CCH):
                            pt = tpp.tile([128, 128], F32, tag="tp")
                            nc.tensor.transpose(pt[:, :], st[:, c0 : c0 + cw], ident[:])
                            nc.vector.tensor_copy(
                                ctb[ci][:, 128 * i : 128 * (i + 1)], pt[:, :]
                            )
                    for j in range(2):
                        ps = qkp.tile([128, 512], F32, tag="qps")
                        for ci in range(6):
                            nc.tensor.matmul(
                                ps[:],
                                wk_t[ci][:, 128 * j : 128 * (j + 1)],
                                ctb[ci][:, :],
                                start=(ci == 0),
                                stop=(ci == 5),
                            )
                        nc.vector.tensor_copy(
                            kT[2 * j][:, 512 * mb : 512 * (mb + 1)], ps[0:64, :]
                        )
                        nc.vector.tensor_copy(
                            kT[2 * j + 1][:, 512 * mb : 512 * (mb + 1)], ps[64:128, :]
                        )
                    for mc in range(4):
                        m = 4 * mb + mc
                        ps = vvp.tile([128, HD], F32, tag="vps")
                        for ci in range(6):
                            nc.tensor.matmul(
                                ps[:],
                                ctb[ci][:, 128 * mc : 128 * (mc + 1)],
                                wv_t[ci][:],
                                start=(ci == 0),
                                stop=(ci == 5),
                            )
                        nc.vector.tensor_copy(
                            v3[m][:, :, 0:DH],
                            ps[:].rearrange("p (h d) -> p h d", h=NHL),
                        )
                        nc.vector.tensor_copy(v3[m][:, :, DH:128], vones[:])

            # ---- attention
            outT = [
                pp.tile([128, N], F32R, tag=f"outT{j}", name=f"outT{j}") for j in range(2)
            ]
            with (
                tc.tile_pool(name="scp", bufs=2, space="PSUM") as scp,
                tc.tile_pool(name="ovp", bufs=2, space="PSUM") as ovp,
                tc.tile_pool(name="expp", bufs=1) as expp,
                tc.tile_pool(name="recp", bufs=2) as recp,
            ):
                for h in range(NHL):
                    for nb in range(N_NB):
                        n0 = NB * nb
                        expT = [
                            expp.tile([128, NB], F32R, tag=f"exp{m}", name=f"exp{m}")
                            for m in range(MCH)
                        ]
                        for m in range(MCH):
                            sc = scp.tile([128, NB], F32, tag="sc")
                            for j in range(NB // 512):
                                nc.tensor.matmul(
                                    sc[:, 512 * j : 512 * (j + 1)],
                                    kT[h][:, 128 * m : 128 * (m + 1)],
                                    qT[h][:, n0 + 512 * j : n0 + 512 * (j + 1)],
                                    start=True,
                                    stop=True,
                                )
                            nc.scalar.activation(
                                expT[m][:],
                                sc[:],
                                mybir.ActivationFunctionType.Exp,
                                scale=SCALE,
                            )
                        ov = ovp.tile([128, NB], F32, tag="ov")
                        for m in range(MCH):
                            for j in range(NB // 512):
                                nc.tensor.matmul(
                                    ov[:, 512 * j : 512 * (j + 1)],
                                    v3[m][:, h, :],
                                    expT[m][:, 512 * j : 512 * (j + 1)],
                                    start=(m == 0),
                                    stop=(m == MCH - 1),
                                )
                        rc = recp.tile([64, NB], F32, tag="rc")
                        nc.vector.reciprocal(rc[:], ov[64:128, :])
                        nc.vector.tensor_mul(
                            outT[h // 2][64 * (h % 2) : 64 * (h % 2) + 64, n0 : n0 + NB],
                            ov[0:64, :],
                            rc[:],
                        )

            # ---- output projection + bias
            with (
                tc.tile_pool(name="fop", bufs=4, space="PSUM") as fop,
                tc.tile_pool(name="fot", bufs=4) as fotp,
            ):
                for i in range(N // 128):
                    fo = fop.tile([128, E], F32, tag="fo")
                    for j in range(2):
                        nc.tensor.matmul(
                            fo[:],
                            outT[j][:, 128 * i : 128 * (i + 1)],
                            wo_t[j][:],
                            start=(j == 0),
                            stop=(j == 1),
                        )
                    ft = fotp.tile([128, E], F32, tag="ft")
                    nc.vector.tensor_add(ft[:], fo[:], bo_b[:])
                    nc.sync.dma_start(out=out_d[128 * i : 128 * (i + 1), :], in_=ft[:])

    nc.compile()
    return nc


def _get_nc(reps=1):
    if reps not in _NC_CACHE:
        _NC_CACHE[reps] = build_nc(reps)
    return _NC_CACHE[reps]


def _in_maps(x, context, Wq, Wk, Wv, Wo, bo):
    x = np.asarray(x, dtype=np.float32)
    context = np.asarray(context, dtype=np.float32)
    Wq = np.asarray(Wq, dtype=np.float32)
    Wk = np.asarray(Wk, dtype=np.float32)
    Wv = np.asarray(Wv, dtype=np.float32)
    Wo = np.asarray(Wo, dtype=np.float32)
    bo = np.asarray(bo, dtype=np.float32)
    zeros_bo = np.zeros_like(bo)
    maps = []
    for c in range(8):
        b, g = c // 2, c % 2
        sl = slice(HD * g, HD * (g + 1))
        maps.append(
            {
                "x": np.ascontiguousarray(x[b]),
                "ctx": np.ascontiguousarray(context[b]),
                "wq": np.ascontiguousarray(Wq[:, sl]),
                "wk": np.ascontiguousarray(Wk[:, sl]),
                "wv": np.ascontiguousarray(Wv[:, sl]),
                "wo": np.ascontiguousarray(Wo[sl, :]),
                "bo": bo if g == 0 else zeros_bo,
            }
        )
    return maps


def run_spmd(inputs, trace=False):
    nc = _get_nc()
    maps = _in_maps(**inputs)
    return run_bass_kernel_spmd(nc, maps, core_ids=list(range(8)), trace=trace)


def kernel(x, context, Wq, Wk, Wv, Wo, bo):
    res = run_spmd(dict(x=x, context=context, Wq=Wq, Wk=Wk, Wv=Wv, Wo=Wo, bo=bo))
    B = np.asarray(x).shape[0]
    out = np.empty((B, N, E), dtype=np.float32)
    for b in range(B):
        out[b] = res.results[2 * b]["out"] + res.results[2 * b + 1]["out"]
    return out
